# revision 5
# baseline (speedup 1.0000x reference)
"""Trainium2 Bass kernel for nn_CrossModalHypergraphPerception.

Sharding: 8 cores; core c handles batch b=c//2, target-row half h=c%2
(2048 of 4096 target nodes). Device computes, per core:
  score = 2*xt.xc - |xc|^2   (bf16 hi/lo split, 3 matmuls per 128-k-tile)
  knn   = per-row top-8 via DVE MAX8/FIND_INDEX8
  Xn_c  = Xc@W1+b1 (row-major bf16, staged to DRAM)
  X_edge= Xn_t + sum_k Xn_c[knn[:,k]]  (dma_gather + DVE tree add)
  X_et  = X_edge @ (W2/9) + b2  (PE transpose + fp32 matmul)
Host merges: out_t from X_et; out_c via segment-sum of X_et over knn
(device scatter-add RMW races on duplicate indices, so the tiny
segment-sum reduction is done host-side), plus exact re-fix of any rows
where hardware FIND_INDEX8 returned duplicate indices on exact ties.
"""
import os
os.environ.setdefault("JAX_PLATFORMS", "")

import numpy as np
import ml_dtypes

B, C, HH, WW = 4, 256, 64, 64
N = HH * WW            # 4096 targets per batch
NC = 2 * N             # 8192 context nodes per batch
KN = 8                 # neighbors
NT = N // 2            # 2048 targets per core
MT = NT // 128         # 16 m-tiles
NTI = NC // 512        # 16 n-tiles
NCORES = 8

BF16 = ml_dtypes.bfloat16

_compiled = {}


def _split_bf16(x):
    x = np.ascontiguousarray(x, dtype=np.float32)
    hi = x.astype(BF16)
    lo = (x - hi.astype(np.float32)).astype(BF16)
    return hi, lo


def _build_nc():
    import concourse.bass as bass
    import concourse.bacc as bacc
    import concourse.mybir as mybir
    import concourse.tile as tile
    from concourse.masks import make_identity

    f32 = mybir.dt.float32
    bf16 = mybir.dt.bfloat16
    i16 = mybir.dt.int16
    u16 = mybir.dt.uint16

    nc = bacc.Bacc("TRN2", target_bir_lowering=False, debug=False,
                   num_devices=NCORES)

    # ---- DRAM I/O ----
    d_xt2h = nc.dram_tensor("xt2h", [C, NT], bf16, kind="ExternalInput")
    d_xt2l = nc.dram_tensor("xt2l", [C, NT], bf16, kind="ExternalInput")
    d_xtnh = nc.dram_tensor("xtnh", [C, NT], bf16, kind="ExternalInput")
    d_xtnl = nc.dram_tensor("xtnl", [C, NT], bf16, kind="ExternalInput")
    d_xch = nc.dram_tensor("xch", [C, NC], bf16, kind="ExternalInput")
    d_xcl = nc.dram_tensor("xcl", [C, NC], bf16, kind="ExternalInput")
    d_nrm = nc.dram_tensor("nrm", [3, NC], bf16, kind="ExternalInput")
    d_w1h = nc.dram_tensor("w1h", [C, C], bf16, kind="ExternalInput")
    d_w1l = nc.dram_tensor("w1l", [C, C], bf16, kind="ExternalInput")
    d_b1 = nc.dram_tensor("b1s", [2, C], bf16, kind="ExternalInput")
    d_w2 = nc.dram_tensor("w2s", [C, C], f32, kind="ExternalInput")
    d_b2 = nc.dram_tensor("b2s", [1, C], f32, kind="ExternalInput")

    d_xet = nc.dram_tensor("xet", [NT, C], f32, kind="ExternalOutput")
    d_knn = nc.dram_tensor("knn", [NT, KN], u16, kind="ExternalOutput")
    d_xnc = nc.dram_tensor("xnc", [NC, C], f32, kind="Internal")

    with tile.TileContext(nc) as tc:
        with tc.tile_pool(name="persist", bufs=1) as pp, \
             tc.tile_pool(name="scores", bufs=2) as scp, \
             tc.tile_pool(name="mt", bufs=2) as mtp, \
             tc.tile_pool(name="gat", bufs=1) as gp, \
             tc.tile_pool(name="d2ps", bufs=5, space="PSUM") as d2ps, \
             tc.tile_pool(name="smps", bufs=3, space="PSUM") as smps:

            # ---- persistent loads ----
            xch = [pp.tile([128, NC], bf16, tag=f"xch{k}", name=f"xch{k}") for k in range(2)]
            xcl = [pp.tile([128, NC], bf16, tag=f"xcl{k}", name=f"xcl{k}") for k in range(2)]
            for k in range(2):
                nc.sync.dma_start(xch[k][:], d_xch[128 * k:128 * (k + 1), :])
                nc.sync.dma_start(xcl[k][:], d_xcl[128 * k:128 * (k + 1), :])
            nrm = pp.tile([3, NC], bf16, tag="nrm")
            nc.sync.dma_start(nrm[:], d_nrm[:, :])
            w1h = [pp.tile([128, C], bf16, tag=f"w1h{k}", name=f"w1h{k}") for k in range(2)]
            w1l = [pp.tile([128, C], bf16, tag=f"w1l{k}", name=f"w1l{k}") for k in range(2)]
            w2 = [pp.tile([128, C], f32, tag=f"w2{k}", name=f"w2{k}") for k in range(2)]
            for k in range(2):
                nc.sync.dma_start(w1h[k][:], d_w1h[128 * k:128 * (k + 1), :])
                nc.sync.dma_start(w1l[k][:], d_w1l[128 * k:128 * (k + 1), :])
                nc.sync.dma_start(w2[k][:], d_w2[128 * k:128 * (k + 1), :])
            b1 = pp.tile([2, C], bf16, tag="b1")
            nc.sync.dma_start(b1[:], d_b1[:, :])
            b2 = pp.tile([1, C], f32, tag="b2")
            nc.sync.dma_start(b2[:], d_b2[:, :])
            ones2 = pp.tile([2, 128], bf16, tag="ones2")
            nc.gpsimd.memset(ones2[:], 1.0)
            ones3 = pp.tile([3, 128], bf16, tag="ones3")
            nc.gpsimd.memset(ones3[:], 1.0)
            ones1f = pp.tile([1, 128], f32, tag="ones1f")
            nc.gpsimd.memset(ones1f[:], 1.0)
            ident = pp.tile([128, 128], f32, tag="ident")
            make_identity(nc, ident[:])
            xedgeT = [pp.tile([128, NT], f32, tag=f"xedgeT{k}", name=f"xedgeT{k}")
                      for k in range(2)]

            # ---- stage 2: Xn_c (row-major, bf16, staged to DRAM) ----
            for j in range(NC // 128):
                q = smps.tile([128, C], f32, tag="sm")
                k0 = j * 128
                for k in range(2):
                    nc.tensor.matmul(q[:], xch[k][:, k0:k0 + 128], w1h[k][:],
                                     start=(k == 0), stop=False)
                for k in range(2):
                    nc.tensor.matmul(q[:], xch[k][:, k0:k0 + 128], w1l[k][:],
                                     start=False, stop=False)
                for k in range(2):
                    nc.tensor.matmul(q[:], xcl[k][:, k0:k0 + 128], w1h[k][:],
                                     start=False, stop=False)
                nc.tensor.matmul(q[:], ones2[:], b1[:], start=False, stop=True)
                st = mtp.tile([128, C], f32, tag="xncst")
                nc.scalar.copy(st[:], q[:])
                nc.sync.dma_start(d_xnc[k0:k0 + 128, :], st[:])

            # ---- per-m-tile back half (emitted with 2-tile lag) ----
            def back(t):
                t0 = t * 128
                # gather idx: knn rows -> wrapped [128, 64] int16 (8x repl)
                gidx = mtp.tile([128, 64], i16, tag="gidx")
                wrap_src = d_knn[t0:t0 + 128, :].bitcast(i16).rearrange(
                    "(c p) k -> p k c", p=16)
                for g in range(8):
                    nc.sync.dma_start(
                        gidx[16 * g:16 * (g + 1), :].rearrange(
                            "p (k c) -> p k c", k=8), wrap_src)
                gt = gp.tile([128, KN, C], f32, tag="g")
                nc.gpsimd.dma_gather(
                    gt[:], d_xnc[:, :], gidx[:],
                    num_idxs=1024, num_idxs_reg=1024, elem_size=C)
                # Xn_t into psum
                w = smps.tile([128, C], f32, tag="sm")
                nh = [mtp.tile([128, 128], bf16, tag=f"nh{k}", name=f"nh{k}") for k in range(2)]
                nl = [mtp.tile([128, 128], bf16, tag=f"nl{k}", name=f"nl{k}") for k in range(2)]
                for k in range(2):
                    nc.sync.dma_start(nh[k][:],
                                      d_xtnh[128 * k:128 * (k + 1), t0:t0 + 128])
                    nc.sync.dma_start(nl[k][:],
                                      d_xtnl[128 * k:128 * (k + 1), t0:t0 + 128])
                for k in range(2):
                    nc.tensor.matmul(w[:], nh[k][:], w1h[k][:],
                                     start=(k == 0), stop=False)
                for k in range(2):
                    nc.tensor.matmul(w[:], nh[k][:], w1l[k][:],
                                     start=False, stop=False)
                for k in range(2):
                    nc.tensor.matmul(w[:], nl[k][:], w1h[k][:],
                                     start=False, stop=False)
                nc.tensor.matmul(w[:], ones2[:], b1[:], start=False, stop=True)
                # tree add (fp32) + Xn_t
                t1 = mtp.tile([128, 4, C], f32, tag="t1")
                nc.vector.tensor_add(t1[:], gt[:, 0:4, :], gt[:, 4:8, :])
                t2 = mtp.tile([128, 2, C], f32, tag="t2")
                nc.vector.tensor_add(t2[:], t1[:, 0:2, :], t1[:, 2:4, :])
                t3 = mtp.tile([128, C], f32, tag="t3")
                nc.vector.tensor_add(t3[:], t2[:, 0:1, :].rearrange("p a c -> p (a c)"),
                                     t2[:, 1:2, :].rearrange("p a c -> p (a c)"))
                xe = mtp.tile([128, C], f32, tag="xe")
                nc.vector.tensor_add(xe[:], t3[:], w[:])
                # transpose X_edge -> xedgeT columns t0:t0+128
                for cch in range(2):
                    tp = smps.tile([128, 128], f32, tag="sm")
                    nc.tensor.transpose(tp[:], xe[:, 128 * cch:128 * (cch + 1)],
                                        ident[:])
                    nc.scalar.copy(xedgeT[cch][:, t0:t0 + 128], tp[:])
                # X_et = X_edgeT.T @ W2s + b2   (fp32)
                x = smps.tile([128, C], f32, tag="sm")
                for k in range(2):
                    nc.tensor.matmul(x[:], xedgeT[k][:, t0:t0 + 128], w2[k][:],
                                     start=(k == 0), stop=False)
                nc.tensor.matmul(x[:], ones1f[:], b2[:], start=False, stop=True)
                xo = mtp.tile([128, C], f32, tag="xo")
                nc.vector.tensor_copy(xo[:], x[:])
                nc.sync.dma_start(d_xet[t0:t0 + 128, :], xo[:])

            # ---- stage 1: d2 + topk, interleaved with back() ----
            for t in range(MT):
                t0 = t * 128
                th = [mtp.tile([128, 128], bf16, tag=f"th{k}", name=f"th{k}") for k in range(2)]
                tl = [mtp.tile([128, 128], bf16, tag=f"tl{k}", name=f"tl{k}") for k in range(2)]
                for k in range(2):
                    nc.sync.dma_start(th[k][:],
                                      d_xt2h[128 * k:128 * (k + 1), t0:t0 + 128])
                    nc.sync.dma_start(tl[k][:],
                                      d_xt2l[128 * k:128 * (k + 1), t0:t0 + 128])
                sc = scp.tile([128, NC], f32, tag="sc")
                for n in range(NTI):
                    n0 = n * 512
                    p = d2ps.tile([128, 512], f32, tag="d2")
                    for k in range(2):
                        nc.tensor.matmul(p[:], th[k][:], xch[k][:, n0:n0 + 512],
                                         start=(k == 0), stop=False)
                    for k in range(2):
                        nc.tensor.matmul(p[:], th[k][:], xcl[k][:, n0:n0 + 512],
                                         start=False, stop=False)
                    for k in range(2):
                        nc.tensor.matmul(p[:], tl[k][:], xch[k][:, n0:n0 + 512],
                                         start=False, stop=False)
                    nc.tensor.matmul(p[:], ones3[:], nrm[:, n0:n0 + 512],
                                     start=False, stop=True)
                    # PSUM -> SBUF score copy; DMA cannot read PSUM, so
                    # split the copies between ScalarE and VectorE.
                    if n % 16 < 10:
                        nc.scalar.copy(sc[:, n0:n0 + 512], p[:])
                    else:
                        nc.vector.tensor_copy(sc[:, n0:n0 + 512], p[:])
                mx = mtp.tile([128, KN], f32, tag="mx")
                nc.vector.max(mx[:], sc[:])
                ix = mtp.tile([128, KN], u16, tag="ix")
                nc.vector.max_index(ix[:], mx[:], sc[:])
                nc.sync.dma_start(d_knn[t0:t0 + 128, :], ix[:])
                if t >= 2:
                    back(t - 2)
            back(MT - 2)
            back(MT - 1)

    nc.compile()
    return nc


def _host_prep(Xc_b):
    """Per-batch context-side input dict. Xc_b: [C, NC] fp32."""
    xch, xcl = _split_bf16(Xc_b)
    s = np.sum(Xc_b.astype(np.float32) ** 2, axis=0, dtype=np.float32)
    m = -s
    n1 = m.astype(BF16)
    r1 = m - n1.astype(np.float32)
    n2 = r1.astype(BF16)
    n3 = (r1 - n2.astype(np.float32)).astype(BF16)
    nrm = np.stack([n1, n2, n3], axis=0)
    return dict(xch=xch, xcl=xcl, nrm=nrm)


def _segment_sum(knn_b, xet_b):
    """contrib[c] = sum over pairs (i,k) with knn[i,k]==c of xet[i]; counts."""
    flat = knn_b.reshape(-1).astype(np.int64)
    counts = np.bincount(flat, minlength=NC)
    order = np.argsort(flat, kind="stable")
    src = xet_b[order // KN]
    sorted_idx = flat[order]
    starts = np.flatnonzero(np.r_[True, sorted_idx[1:] != sorted_idx[:-1]])
    sums = np.add.reduceat(src, starts, axis=0)
    contrib = np.zeros((NC, C), np.float32)
    contrib[sorted_idx[starts]] = sums
    return contrib, counts


def _fix_dup_rows(knn_b, xet_b, Xt_nodes, Xc_nodes, W1, b1, W2, b2):
    """Exact recompute of rows where the device top-8 contains duplicate
    indices (hardware FIND_INDEX8 latches the same position for tied
    values)."""
    dup = np.zeros(knn_b.shape[0], bool)
    srt = np.sort(knn_b.astype(np.int64), axis=1)
    dup |= (srt[:, 1:] == srt[:, :-1]).any(axis=1)
    rows = np.flatnonzero(dup)
    if rows.size == 0:
        return
    nc2 = np.sum(Xc_nodes.astype(np.float32) ** 2, axis=1)
    for i in rows:
        xt = Xt_nodes[i]
        d2 = nc2 - 2.0 * (Xc_nodes @ xt)
        idx = np.argsort(d2, kind="stable")[:KN]
        knn_b[i] = idx
        xn_t = xt @ W1 + b1
        xn_nb = Xc_nodes[idx] @ W1 + b1
        x_edge = (xn_t + xn_nb.sum(axis=0)) / np.float32(KN + 1)
        xet_b[i] = x_edge @ W2 + b2


def kernel(X_target, X_context1, X_context2, W1, b1, W2, b2):
    from concourse.bass_utils import run_bass_kernel_spmd

    X_target = np.asarray(X_target, np.float32)
    X_context1 = np.asarray(X_context1, np.float32)
    X_context2 = np.asarray(X_context2, np.float32)
    W1 = np.asarray(W1, np.float32)
    b1 = np.asarray(b1, np.float32)
    W2 = np.asarray(W2, np.float32)
    b2 = np.asarray(b2, np.float32)

    if "nc" not in _compiled:
        _compiled["nc"] = _build_nc()
    nc = _compiled["nc"]

    XtT = X_target.reshape(B, C, N)                       # [B, C, N]
    XcT = np.concatenate([X_context1.reshape(B, C, N),
                          X_context2.reshape(B, C, N)], axis=2)  # [B, C, NC]

    w1h, w1l = _split_bf16(W1)
    b1h, b1l = _split_bf16(b1)
    b1s = np.stack([b1h, b1l], axis=0)
    w2s = np.ascontiguousarray(W2 / np.float32(KN + 1))
    b2s = b2.reshape(1, C).astype(np.float32)

    shared = dict(w1h=w1h, w1l=w1l, b1s=b1s, w2s=w2s, b2s=b2s)
    batch_prep = []
    for b in range(B):
        batch_prep.append(_host_prep(XcT[b]))

    in_maps = []
    for c in range(NCORES):
        b, h = c // 2, c % 2
        Xt_half = np.ascontiguousarray(XtT[b][:, h * NT:(h + 1) * NT])
        xt2h, xt2l = _split_bf16(2.0 * Xt_half)
        xtnh, xtnl = _split_bf16(Xt_half)
        m = dict(batch_prep[b])
        m.update(xt2h=xt2h, xt2l=xt2l, xtnh=xtnh, xtnl=xtnl)
        m.update(shared)
        in_maps.append(m)

    res = run_bass_kernel_spmd(nc, in_maps, core_ids=list(range(NCORES)))
    if res.exec_time_ns is not None:
        print(f"HW exec time: {res.exec_time_ns} ns")

    out_t = np.empty((B, C, HH, WW), np.float32)
    out_c1 = np.empty((B, C, HH, WW), np.float32)
    out_c2 = np.empty((B, C, HH, WW), np.float32)
    for b in range(B):
        xet_b = np.concatenate([res.results[2 * b]["xet"],
                                res.results[2 * b + 1]["xet"]], axis=0)
        knn_b = np.concatenate([res.results[2 * b]["knn"],
                                res.results[2 * b + 1]["knn"]],
                               axis=0).astype(np.int64)
        Xt_nodes = XtT[b].T        # [N, C]
        Xc_nodes = XcT[b].T        # [NC, C]
        _fix_dup_rows(knn_b, xet_b, Xt_nodes, Xc_nodes, W1, b1, W2, b2)
        contrib, counts = _segment_sum(knn_b, xet_b)
        out_c = contrib / np.clip(counts, 1, None)[:, None].astype(np.float32)
        out_t[b] = xet_b.T.reshape(C, HH, WW)
        out_c1[b] = out_c[:N].T.reshape(C, HH, WW)
        out_c2[b] = out_c[N:].T.reshape(C, HH, WW)
    return (out_t, out_c1, out_c2)
